# revision 1
# baseline (speedup 1.0000x reference)
"""CompoundLoss (dice + focal + edge) Trainium2 Bass kernel.

Self-contained: hardcodes shapes [8,11,512,512] f32 logits + [8,512,512] i32
targets, shards batch across 8 NeuronCores (pure data parallel). Each core
reduces its image to 94 per-partition fp32 accumulator columns; the host
finishes the tiny scalar math in fp64.

Math notes (per image):
  softmax: E_c = exp(L_c) (bf16), Z = sum_c E_c (PE identity-matmul accumulate
  into PSUM, fp32), r = exp(-ln Z) via ACT. pt = (sum_c [T==c]*E_c) * r.
  dice: inter[c] = sum_p [T==c]*pt, card[c] = sum_p E_c*r + count[c] + eps.
  focal: mean(-0.25*(1-pt)^2*ln(pt)).
  edge: multi-hot preds pm_c = [E_c >= max_c E_c] packed as PB = sum 2^c*pm_c
  (PE scaled-identity accumulate). Targets packed as one-hot bitmask
  bm = 1<<T (int16). 3x3 OR/AND window convs + 4-neighbor OR via shifted-AP
  TTs and partition-shift DMAs give bmOR (class presence in window), bmAND
  (uniform window), bm4 (4-neighbor presence). Then per class c=1..10:
    num[c] = y1 - y2 + es2*y3 + (e1-es2)*y4,  den[c] = denP - denN
    y1 = #bit_c(bm & ~PB)    (single-bit plane -> is_equal-2^c + fused accum)
    y2 = #bit_c(bmAND & ~PB) (single-bit)
    y3 = #bit_c(PB & ~bm & bmOR), y4 = #bit_c(PB & ~bm & bm4)  (2-op extract)
    denP = #bit_c(bmOR) (2-op), denN = #bit_c(bmAND) (single-bit)
"""

import numpy as np

B, C, H, W = 8, 11, 512, 512
P = 128
KB = H // P          # 4 row-blocks
NF = KB * W          # 2048 free elems per partition
NPIX = H * W
EPS = 1e-6
E1 = float(np.exp(-1.0))
ES2 = float(np.exp(-np.sqrt(2.0)))

# stats column layout
COL_INTER = 0          # 11
COL_SUMP = 11          # 11
COL_COUNT = 22         # 11
COL_FOCAL = 33         # 1
COL_Y1 = 34            # 10 (c=1..10)
COL_Y2 = 44
COL_Y3 = 54
COL_Y4 = 64
COL_DENP = 74
COL_DENN = 84
NCOL = 94

_cache = {}


def _build():
    import ml_dtypes
    import concourse.bacc as bacc
    import concourse.mybir as mybir
    from concourse.tile import TileContext

    f32 = mybir.dt.float32
    bf16 = mybir.dt.bfloat16
    i32 = mybir.dt.int32
    i16 = mybir.dt.int16
    op = mybir.AluOpType
    act = mybir.ActivationFunctionType

    nc = bacc.Bacc()
    x = nc.dram_tensor("x", [C, H, W], f32, kind="ExternalInput")
    t = nc.dram_tensor("t", [H, W], i32, kind="ExternalInput")
    stats_out = nc.dram_tensor("stats", [P, NCOL], f32, kind="ExternalOutput")
    statsa_out = nc.dram_tensor("statsa", [P, 32], f32, kind="ExternalOutput")
    statsp_out = nc.dram_tensor("statsp", [P, 4], f32, kind="ExternalOutput")

    # [C, 128, 4, 512] view: row = 128*k + p
    xv = x[:, :, :].rearrange("c (k p) w -> c p k w", p=P)
    tv = t[:, :].rearrange("(k p) w -> p k w", p=P)

    ident_np = np.eye(P, dtype=np.float32)
    ident_d = nc.inline_tensor(ident_np.astype(ml_dtypes.bfloat16), name="ident")
    # scaled identities 2^c for PB accumulation, c=1..10
    sid_np = np.stack([ident_np * float(1 << c) for c in range(1, 11)])
    sid_d = nc.inline_tensor(sid_np.astype(ml_dtypes.bfloat16), name="sident")
    # ones-column matrices: ocol_c[p, m] = [m == c]; ones-mm routes the
    # column-sum of rhs into PSUM partition-row c (zeros elsewhere)
    ocol_np = np.zeros((C, P, P), dtype=np.float32)
    for c_ in range(C):
        ocol_np[c_, :, c_] = 1.0
    ocol_d = nc.inline_tensor(ocol_np.astype(ml_dtypes.bfloat16), name="ocol")

    with TileContext(nc, pool_alloc_mode="queue") as tc:
        with (
            tc.tile_pool(name="persist", bufs=1) as pp,
            tc.tile_pool(name="scratch", bufs=2) as sp,
        ):
            # constants
            identb = pp.tile([P, P], bf16, name="identb")
            nc.sync.dma_start(identb, ident_d[:, :])
            sids = []
            for ci in range(10):
                sid = pp.tile([P, P], bf16, name=f"sid{ci}", tag=f"sid{ci}")
                nc.sync.dma_start(sid, sid_d[ci])
                sids.append(sid)
            ocols = []
            for ci in range(C):
                oc = pp.tile([P, P], bf16, name=f"oc{ci}", tag=f"oc{ci}")
                nc.sync.dma_start(oc, ocol_d[ci])
                ocols.append(oc)

            stats = pp.tile([P, NCOL], f32, name="stats")
            statsa = pp.tile([P, 32], f32, name="statsa")
            statsp = pp.tile([P, 4], f32, name="statsp")

            # targets
            ts32 = sp.tile([P, NF], i32, name="ts32", tag="f32s", bufs=1)
            nc.sync.dma_start(ts32.rearrange("p (k w) -> p k w", w=W), tv)
            t16b = pp.tile([P, NF], bf16, name="t16b")
            nc.vector.tensor_copy(t16b, ts32)
            t16i = pp.tile([P, NF], i16, name="t16i")
            nc.vector.tensor_copy(t16i, ts32)

            # ---- phase A: load logits, exp, Z accumulation in PSUM ----
            E = []
            _lcm = tc.tile_pool(name="lpool", bufs=2)
            _ecm = tc.tile_pool(name="epool", bufs=11)
            _mcm = tc.tile_pool(name="mxpool", bufs=4)
            _ccm2 = tc.tile_pool(name="cpool", bufs=2)
            lpool = _lcm.__enter__()
            epool = _ecm.__enter__()
            mxp = _mcm.__enter__()
            cp = _ccm2.__enter__()
            with tc.tile_pool(name="zpsum", bufs=1, space="PSUM") as zp:
                zps = zp.tile([P, NF], f32, name="zps")
                for c in range(C):
                    lb = lpool.tile([P, NF], f32, name=f"lb{c}", tag="lb")
                    nc.sync.dma_start(
                        lb.rearrange("p (k w) -> p k w", w=W), xv[c]
                    )
                    e = epool.tile([P, NF], bf16, name=f"e{c}", tag="e")
                    nc.scalar.activation(e, lb, act.Exp)
                    E.append(e)
                    for k in range(KB):
                        nc.tensor.matmul(
                            zps[:, k * W : (k + 1) * W],
                            identb,
                            e[:, k * W : (k + 1) * W],
                            start=(c == 0),
                            stop=(c == C - 1),
                        )
                # r = exp(-ln Z)
                lnz = sp.tile([P, NF], f32, name="lnz", tag="f32s", bufs=1)
                nc.scalar.activation(lnz, zps, act.Ln)
            r = pp.tile([P, NF], bf16, name="r")
            nc.scalar.activation(r, lnz, act.Exp, scale=-1.0)

            # ---- Emax tree (bf16), depth ~4 with 4 rotating slots ----
            emax = pp.tile([P, NF], bf16, name="emax")

            def vmax(a, b, nm):
                o = mxp.tile([P, NF], bf16, name=nm, tag="mx")
                nc.vector.tensor_tensor(o, a, b, op.max)
                return o

            m01 = vmax(E[0], E[1], "m01")
            m23 = vmax(E[2], E[3], "m23")
            h0 = vmax(m01, m23, "h0")      # frees 2 slots
            m45 = vmax(E[4], E[5], "m45")
            m67 = vmax(E[6], E[7], "m67")
            h1 = vmax(m45, m67, "h1")
            m89 = vmax(E[8], E[9], "m89")
            h2 = vmax(m89, E[10], "h2")
            h3 = vmax(h0, h1, "h3")
            nc.vector.tensor_tensor(emax, h3, h2, op.max)

            # ---- phase C: per-class products; pt/sumP/inter/PB via PE ----
            with (
                tc.tile_pool(name="ptpsum", bufs=1, space="PSUM") as ptp,
                tc.tile_pool(name="hpsum", bufs=1, space="PSUM") as hp,
            ):
                ptps = ptp.tile([P, NF], f32, name="ptps")
                spbank = hp.tile([P, W], f32, name="spbank")
                inbank = hp.tile([P, W], f32, name="inbank")
                for c in range(C):
                    # oh_c = [T == c] (bf16 0/1), fused count accum
                    oh = cp.tile([P, NF], bf16, name=f"oh{c}", tag="oh", bufs=2)
                    nc.vector.tensor_scalar(
                        oh, t16i, c, 0.0, op.is_equal, op.add,
                        accum_out=stats[:, COL_COUNT + c : COL_COUNT + c + 1],
                    )
                    # P_c = E_c * r (softmax probs, bf16)
                    pc = cp.tile([P, NF], bf16, name=f"pc{c}", tag="pc", bufs=2)
                    nc.vector.tensor_tensor(pc, E[c], r, op.mult)
                    # ohP_c = oh_c * P_c
                    ohp = cp.tile([P, NF], bf16, name=f"ohp{c}", tag="ohp", bufs=2)
                    nc.vector.tensor_tensor(ohp, oh, pc, op.mult)
                    for k in range(KB):
                        sl = slice(k * W, (k + 1) * W)
                        # sumP[c] += colsum(P_c) into spbank row c
                        nc.tensor.matmul(
                            spbank[:, :], ocols[c], pc[:, sl],
                            start=(c == 0 and k == 0),
                            stop=(c == C - 1 and k == KB - 1),
                        )
                        # inter[c] += colsum(ohP_c) into inbank row c
                        nc.tensor.matmul(
                            inbank[:, :], ocols[c], ohp[:, sl],
                            start=(c == 0 and k == 0),
                            stop=(c == C - 1 and k == KB - 1),
                        )
                        # pt += ohP_c (identity accumulate)
                        nc.tensor.matmul(
                            ptps[:, sl], identb, ohp[:, sl],
                            start=(c == 0),
                            stop=(c == C - 1),
                        )
                # per-class sums: partition-row c of the histo banks
                sp_sc = cp.tile([P, W], f32, name="sp_sc", tag="spsc", bufs=1)
                nc.vector.tensor_scalar(
                    sp_sc, spbank, 1, 0.0, op.mult, op.add,
                    accum_out=statsp[:, 0:1],
                )
                in_sc = cp.tile([P, W], f32, name="in_sc", tag="insc", bufs=1)
                nc.vector.tensor_scalar(
                    in_sc, inbank, 1, 0.0, op.mult, op.add,
                    accum_out=statsp[:, 1:2],
                )
                # pt plane (bf16) via ACT copy from PSUM
                pt = pp.tile([P, NF], bf16, name="pt")
                nc.scalar.copy(pt, ptps)

            # ---- PB: multi-hot pred bitmask via scaled-identity matmuls ----
            pb16 = pp.tile([P, NF], i16, name="pb16")
            with tc.tile_pool(name="pbpsum", bufs=1, space="PSUM") as pbp:
                pbps = pbp.tile([P, NF], f32, name="pbps")
                for c in range(1, C):
                    pm = cp.tile([P, NF], bf16, name=f"pm{c}", tag="pm", bufs=2)
                    nc.vector.tensor_tensor(pm, E[c], emax, op.is_ge)
                    for k in range(KB):
                        nc.tensor.matmul(
                            pbps[:, k * W : (k + 1) * W],
                            sids[c - 1],
                            pm[:, k * W : (k + 1) * W],
                            start=(c == 1),
                            stop=(c == C - 1),
                        )
                # PB -> int16 (values are exact small ints in fp32 PSUM)
                nc.vector.tensor_copy(pb16, pbps)


            # ---- focal ----
            lg = cp.tile([P, NF], bf16, name="lg", tag="lg", bufs=1)
            nc.scalar.activation(lg, pt, act.Ln)
            q = cp.tile([P, NF], bf16, name="q", tag="q", bufs=1)
            nc.vector.tensor_scalar(q, pt, -1.0, 1.0, op.mult, op.add)
            q2 = cp.tile([P, NF], bf16, name="q2", tag="q2", bufs=1)
            nc.scalar.square(q2, q)
            fsc = cp.tile([P, NF], bf16, name="fsc", tag="fsc", bufs=1)
            nc.vector.scalar_tensor_tensor(
                fsc, q2, 1.0, lg, op.mult, op.mult,
                accum_out=stats[:, COL_FOCAL : COL_FOCAL + 1],
            )




            _ccm2.__exit__(None, None, None)
            _mcm.__exit__(None, None, None)
            _ecm.__exit__(None, None, None)
            _lcm.__exit__(None, None, None)
            # ---- edge: bitmask planes (int16) ----
            _ccm = tc.tile_pool(name="convp", bufs=1)
            convp = _ccm.__enter__()
            NPAD = KB * (W + 2)  # padded [128, 4, 514]
            bmp = convp.tile([P, NPAD], i16, name="bmp")
            nc.gpsimd.memset(bmp, 0)
            bmp3 = bmp.rearrange("p (k w) -> p k w", w=W + 2)
            bm_c = bmp3[:, :, 1 : W + 1]     # center view [128,4,512]
            bm_l = bmp3[:, :, 0:W]
            bm_r = bmp3[:, :, 2 : W + 2]
            ones16 = convp.tile([P, NF], i16, name="ones16", tag="ones16", bufs=1)
            nc.gpsimd.memset(ones16, 1)
            t16i3 = t16i.rearrange("p (k w) -> p k w", w=W)
            nc.vector.tensor_tensor(bm_c, ones16.rearrange("p (k w) -> p k w", w=W),
                                    t16i3, op.logical_shift_left)

            def i16tile(nm, pool=None):
                return (pool or convp).tile([P, NF], i16, name=nm)

            v3 = lambda a: a.rearrange("p (k w) -> p k w", w=W)

            bmrow = i16tile("bmrow")   # l | r
            nc.vector.tensor_tensor(v3(bmrow), bm_l, bm_r, op.bitwise_or)
            bmrow3 = i16tile("bmrow3")  # l | c | r
            nc.vector.tensor_tensor(v3(bmrow3), v3(bmrow), bm_c, op.bitwise_or)
            bmar = i16tile("bmar")     # l & r
            nc.vector.tensor_tensor(v3(bmar), bm_l, bm_r, op.bitwise_and)
            bma3 = i16tile("bma3")     # l & c & r
            nc.vector.tensor_tensor(v3(bma3), v3(bmar), bm_c, op.bitwise_and)
            bmc_t = i16tile("bmc_t", pp)   # contiguous copy of center
            nc.vector.tensor_copy(v3(bmc_t), bm_c)

            # vertical shifts via SBUF->SBUF DMA (row +-1 with cross-block stitch)
            def vshift(src, nm_dn, nm_up):
                s3 = v3(src)
                dn = i16tile(nm_dn)
                up = i16tile(nm_up)
                d3, u3 = v3(dn), v3(up)
                nc.gpsimd.memset(dn, 0)
                nc.gpsimd.dma_start(d3[1:P, :, :], s3[0 : P - 1, :, :])
                nc.gpsimd.dma_start(d3[0:1, 1:KB, :], s3[P - 1 : P, 0 : KB - 1, :])
                nc.gpsimd.memset(up, 0)
                nc.gpsimd.dma_start(u3[0 : P - 1, :, :], s3[1:P, :, :])
                nc.gpsimd.dma_start(u3[P - 1 : P, 0 : KB - 1, :], s3[0:1, 1:KB, :])
                return dn, up

            odn, oup = vshift(bmrow3, "odn", "oup")
            adn, aup = vshift(bma3, "adn", "aup")
            bdn, bup = vshift(bmc_t, "bdn", "bup")

            bmor = i16tile("bmor", pp)
            nc.vector.tensor_tensor(bmor, odn, oup, op.bitwise_or)
            nc.vector.tensor_tensor(bmor, bmor, bmrow3, op.bitwise_or)
            bmand = i16tile("bmand", pp)
            nc.vector.tensor_tensor(bmand, adn, aup, op.bitwise_and)
            nc.vector.tensor_tensor(bmand, bmand, bma3, op.bitwise_and)
            bm4 = i16tile("bm4", pp)
            nc.vector.tensor_tensor(bm4, bdn, bup, op.bitwise_or)
            nc.vector.tensor_tensor(bm4, bm4, bmrow, op.bitwise_or)


            _ccm.__exit__(None, None, None)

            _ycm = tc.tile_pool(name="ypool", bufs=2)
            yp = _ycm.__enter__()

            # Y planes
            notpb = yp.tile([P, NF], i16, name="notpb", tag="ych", bufs=3)
            nc.vector.tensor_scalar(notpb, pb16, 0x7FF, None, op.bitwise_xor)
            notbm = yp.tile([P, NF], i16, name="notbm", tag="ych", bufs=3)
            nc.vector.tensor_scalar(notbm, bmc_t, 0x7FF, None, op.bitwise_xor)
            y1p = i16tile("y1p", pp)       # bm & ~PB (single-bit)
            nc.vector.tensor_tensor(y1p, bmc_t, notpb, op.bitwise_and)
            y2p = i16tile("y2p", pp)       # bmAND & ~PB (single-bit)
            nc.vector.tensor_tensor(y2p, bmand, notpb, op.bitwise_and)
            pbnb = yp.tile([P, NF], i16, name="pbnb", tag="ych", bufs=3)     # PB & ~bm
            nc.vector.tensor_tensor(pbnb, pb16, notbm, op.bitwise_and)
            y3p = i16tile("y3p", pp)       # PB & ~bm & bmOR (multi-bit)
            nc.vector.tensor_tensor(y3p, pbnb, bmor, op.bitwise_and)
            y4p = i16tile("y4p", pp)       # PB & ~bm & bm4 (multi-bit)
            nc.vector.tensor_tensor(y4p, pbnb, bm4, op.bitwise_and)


            # per-class accumulations
            def eq_acc(src, val, col, nm):
                # single-bit plane: count of [src == val], fused accum
                o = yp.tile([P, NF], i16, name=nm, tag="sbit", bufs=4)
                nc.vector.tensor_scalar(
                    o, src, val, 0.0, op.is_equal, op.add,
                    accum_out=stats[:, col : col + 1],
                )

            na = [0]

            def bit_acc(src, c, col, nm, acc_engine):
                # multi-bit plane: (src >> c) & 1, then arith accum
                b = yp.tile([P, NF], i16, name=nm, tag="sbit", bufs=4)
                nc.vector.tensor_scalar(
                    b, src, c, 1, op.logical_shift_right, op.bitwise_and
                )
                if acc_engine == "act":
                    oa = yp.tile([P, NF], f32, name=nm + "f", tag="bitf", bufs=1)
                    k = na[0]; na[0] += 1
                    nc.scalar.activation(
                        oa, b, act.Identity,
                        accum_out=statsa[:, k : k + 1],
                    )
                    acol_map.append((col, k))
                else:
                    o = yp.tile([P, NF], i16, name=nm + "a", tag="sbit", bufs=4)
                    nc.vector.tensor_scalar(
                        o, b, 1, 0.0, op.mult, op.add,
                        accum_out=stats[:, col : col + 1],
                    )

            acol_map = []
            for c in range(1, C):
                eq_acc(y1p, 1 << c, COL_Y1 + c - 1, f"by1_{c}")
                eq_acc(y2p, 1 << c, COL_Y2 + c - 1, f"by2_{c}")
                eq_acc(bmand, 1 << c, COL_DENN + c - 1, f"bdn{c}")
                bit_acc(y3p, c, COL_Y3 + c - 1, f"by3_{c}", "act")
                bit_acc(y4p, c, COL_Y4 + c - 1, f"by4_{c}",
                        "act" if c % 2 == 1 else "dve")
                bit_acc(bmor, c, COL_DENP + c - 1, f"bdp{c}",
                        "act" if c % 2 == 0 else "dve")

            nc.gpsimd.dma_start(stats_out[:, :], stats)
            nc.gpsimd.dma_start(statsa_out[:, :], statsa)
            nc.gpsimd.dma_start(statsp_out[:, :], statsp)
            _ycm.__exit__(None, None, None)

    nc.compile()
    return nc, acol_map


def _decode(stats_list):
    """stats_list: 8 arrays [128, NCOL] fp32 -> (total, dice, focal, edge)."""
    dices, focals, edges = [], [], []
    for s in stats_list:
        v = s.astype(np.float64).sum(axis=0)
        inter = v[COL_INTER : COL_INTER + 11]
        sump = v[COL_SUMP : COL_SUMP + 11]
        count = v[COL_COUNT : COL_COUNT + 11]
        dice = (2.0 * inter + EPS) / (sump + count + EPS)
        dices.append(dice.mean())
        focals.append(-0.25 * v[COL_FOCAL] / NPIX)
        ny1 = v[COL_Y1 : COL_Y1 + 10]
        ny2 = v[COL_Y2 : COL_Y2 + 10]
        ny3 = v[COL_Y3 : COL_Y3 + 10]
        ny4 = v[COL_Y4 : COL_Y4 + 10]
        denp = v[COL_DENP : COL_DENP + 10]
        denn = v[COL_DENN : COL_DENN + 10]
        num = (ny1 - ny2) + ES2 * ny3 + (E1 - ES2) * ny4
        den = denp - denn
        cls = np.where(den > 0, num / np.maximum(den, 1.0), 0.0)
        edges.append(cls.mean())
    dice_loss = 1.0 - float(np.mean(dices))
    focal_loss = float(np.mean(focals))
    edge_loss = float(np.mean(edges))
    total = dice_loss + focal_loss + edge_loss
    return (
        np.float32(total),
        np.float32(dice_loss),
        np.float32(focal_loss),
        np.float32(edge_loss),
    )


def kernel(inputs: np.ndarray, targets: np.ndarray):
    from concourse.bass_utils import run_bass_kernel_spmd

    if "nc" not in _cache:
        _cache["nc"], _cache["acol_map"] = _build()
    nc = _cache["nc"]

    inputs = np.ascontiguousarray(np.asarray(inputs, dtype=np.float32))
    targets = np.ascontiguousarray(np.asarray(targets, dtype=np.int32))
    in_maps = [{"x": inputs[b], "t": targets[b]} for b in range(B)]
    res = run_bass_kernel_spmd(nc, in_maps, core_ids=list(range(B)))
    _cache["last_result"] = res
    merged = []
    for rr in res.results:
        s_ = rr["stats"].astype(np.float64).copy()
        sa = rr["statsa"].astype(np.float64)
        for col, k in _cache["acol_map"]:
            s_[:, col] = sa[:, k]
        # sumP[c]/inter[c] live in partition-row c of statsp cols 0/1
        sp_ = rr["statsp"].astype(np.float64)
        s_[:, COL_SUMP : COL_SUMP + 11] = 0.0
        s_[0, COL_SUMP : COL_SUMP + 11] = sp_[0:11, 0]
        s_[:, COL_INTER : COL_INTER + 11] = 0.0
        s_[0, COL_INTER : COL_INTER + 11] = sp_[0:11, 1]
        merged.append(s_)
    return _decode(merged)



# revision 15
# speedup vs baseline: 1.3034x; 1.3034x over previous
"""CompoundLoss (dice + focal + edge) Trainium2 Bass kernel, v2.

Self-contained: hardcodes shapes [8,11,512,512] f32 logits + [8,512,512] i32
targets, shards batch across 8 NeuronCores (pure data parallel). Each core
reduces its image to a few fp32 accumulator columns; the host finishes the
tiny scalar math in fp64.

v2 structure (vs v1): logits stream in as bf16 via casting SWDGE DMAs (halves
DMA bytes into SBUF); target-side mask pipeline (3x3 OR/AND windows via
shifted column views + partition-shift SBUF-SBUF DMAs) overlaps the logit
stream; exact one-hot argmax bitmask PB1 = PB & -PB (lowest set bit = first
maximal class, matching np.argmax tie behavior); edge counts via value-coded
planes + fused is_equal accumulations; eroded (bmand) counts via ACT
sign-telescope; focal fused on ACT directly from the pt PSUM.

Math notes (per image):
  softmax: E_c = exp(L_c) (bf16), Z = sum_c E_c (PE identity-matmul accumulate
  into PSUM, fp32), r = exp(-ln Z). pc = E_c*r, pt = sum_c [T==c]*pc (PE).
  dice: inter[c]/sumP[c] = colsums of ohp/pc via ocol matmuls into PSUM bank
  partition-rows, card[c] = sumP[c] + count[c] + eps.
  focal: mean(-0.25*(1-pt)^2*ln(pt)) with (1-pt)^2 and ln(pt) on ACT.
  edge: bm = 1<<T (i16), PB = sum 2^c*[E_c >= max E] (PE scaled-identity),
  PB1 = PB & -PB. 3x3 windows: row or/and via shifted column views, vertical
  via partition-shift SBUF-SBUF DMAs. Value-coded planes (one-hot-or-zero):
    vA  = bm & (bm^PB1) & ~bmand   -> count[vA==2^c]  = y1-y2
    vB3 = PB1 & (bm^PB1) & bmor    -> count[vB3==2^c] = y3
    vB4 = PB1 & (bm^PB1) & bm4     -> count[vB4==2^c] = y4
    denp[c] = popcount of bit c of bmor (fused shift+and+accum)
    denn[c] via ACT sign-telescope on bmand: S_c = 2*M_c + n_c - NPIX
"""

import numpy as np

B, C, H, W = 8, 11, 512, 512
P = 128
KB = H // P          # 4 row-blocks
NF = KB * W          # 2048 free elems per partition
NPIX = H * W
EPS = 1e-6
E1 = float(np.exp(-1.0))
ES2 = float(np.exp(-np.sqrt(2.0)))

# stats column layout (summed over partitions on host)
SC_COUNT = 0         # 11
SC_FOCAL = 11        # 1
SC_VA = 12           # 10 (c=1..10): y1 - y2
SC_VB3 = 22          # 10: y3
SC_VB4 = 32          # 10: y4
SC_DENP = 42         # 10
NCOL = 52
NACOL = 12           # statsa: denn sign-telescope S_c in cols 0..9

_cache = {}


def _build():
    import ml_dtypes
    import concourse.bacc as bacc
    import concourse.mybir as mybir
    from concourse.tile import TileContext

    f32 = mybir.dt.float32
    bf16 = mybir.dt.bfloat16
    i32 = mybir.dt.int32
    i16 = mybir.dt.int16
    op = mybir.AluOpType
    act = mybir.ActivationFunctionType

    nc = bacc.Bacc()
    x = nc.dram_tensor("x", [C, H, W], f32, kind="ExternalInput")
    t = nc.dram_tensor("t", [H, W], i32, kind="ExternalInput")
    stats_out = nc.dram_tensor("stats", [P, NCOL], f32, kind="ExternalOutput")
    statsa_out = nc.dram_tensor("statsa", [P, NACOL], f32, kind="ExternalOutput")
    statsp_out = nc.dram_tensor("statsp", [P, 4], f32, kind="ExternalOutput")

    # [C, 128, 4, 512] view: row = 128*k + p
    xv = x[:, :, :].rearrange("c (k p) w -> c p k w", p=P)
    tv = t[:, :].rearrange("(k p) w -> p k w", p=P)

    # constants: ident | sid(1..10) | ocol(0..10), one DMA
    ident_np = np.eye(P, dtype=np.float32)
    blocks = [ident_np]
    for c in range(1, C):
        blocks.append(ident_np * float(1 << c))
    ocol_np = np.zeros((C, P, P), dtype=np.float32)
    for c in range(C):
        ocol_np[c, :, c] = 1.0
    blocks.extend(ocol_np)
    cb_np = np.concatenate(blocks, axis=1)           # [128, 22*128]
    cb_d = nc.inline_tensor(cb_np.astype(ml_dtypes.bfloat16), name="cb")

    with TileContext(nc, pool_alloc_mode="queue") as tc:
        with (
            tc.tile_pool(name="persist", bufs=1) as pp,
            tc.tile_pool(name="cpool", bufs=2) as cp,
            tc.tile_pool(name="ypool", bufs=4) as yp,
        ):
            cbt = pp.tile([P, 22 * P], bf16, name="cbt")
            nc.sync.dma_start(cbt, cb_d[:, :])
            identb = cbt[:, 0:P]
            sid = lambda c: cbt[:, c * P:(c + 1) * P]            # c=1..10
            ocol = lambda c: cbt[:, (11 + c) * P:(12 + c) * P]   # c=0..10

            # targets: cast DMA i32 -> i16
            t16i = pp.tile([P, NF], i16, name="t16i")
            nc.gpsimd.dma_start(t16i.rearrange("p (k w) -> p k w", w=W), tv)
            t16i3 = t16i.rearrange("p (k w) -> p k w", w=W)

            stats = pp.tile([P, NCOL], f32, name="stats")
            statsa = pp.tile([P, NACOL], f32, name="statsa")
            statsp = pp.tile([P, 4], f32, name="statsp")

            # bias constants for ACT (per-partition scalars)
            bq = pp.tile([P, 1], f32, name="bq")
            nc.gpsimd.memset(bq, 1.0)
            bsg = pp.tile([P, 10], f32, name="bsg")
            for c in range(1, C):
                nc.gpsimd.memset(bsg[:, c - 1:c], -float(1 << c))

            # persistent mask planes
            NPAD = KB * (W + 2)
            bmp = pp.tile([P, NPAD], i16, name="bmp")
            bmp3 = bmp.rearrange("p (k w) -> p k w", w=W + 2)
            bm_c = bmp3[:, :, 1:W + 1]
            bm_l = bmp3[:, :, 0:W]
            bm_r = bmp3[:, :, 2:W + 2]
            bmor = pp.tile([P, NF], i16, name="bmor")
            bmand = pp.tile([P, NF], i16, name="bmand")
            bm4 = pp.tile([P, NF], i16, name="bm4")
            notbmand = pp.tile([P, NF], i16, name="notbmand")
            emax = pp.tile([P, NF], bf16, name="emax")
            r = pp.tile([P, NF], bf16, name="r")
            Et = pp.tile([P, C * NF], bf16, name="Et")
            E = lambda c: Et[:, c * NF:(c + 1) * NF]

            v3 = lambda a: a.rearrange("p (k w) -> p k w", w=W)

            # ---- target bitmask bm + 3x3 windows ----
            nc.gpsimd.memset(bmp3[:, :, 0:1], 0)
            nc.gpsimd.memset(bmp3[:, :, W + 1:W + 2], 0)
            _mcm = tc.tile_pool(name="maskp", bufs=1)
            mk = _mcm.__enter__()
            ones16 = mk.tile([P, NF], i16, name="ones16")
            nc.gpsimd.memset(ones16, 1)
            nc.vector.tensor_tensor(bm_c, v3(ones16), t16i3,
                                    op.logical_shift_left)
            # row windows (DVE)
            bmrow3 = mk.tile([P, NF], i16, name="bmrow3")
            nc.vector.tensor_tensor(v3(bmrow3), bm_l, bm_r, op.bitwise_or)
            nc.vector.tensor_tensor(v3(bmrow3), v3(bmrow3), bm_c, op.bitwise_or)
            bma3 = mk.tile([P, NF], i16, name="bma3")
            nc.vector.tensor_tensor(v3(bma3), bm_l, bm_r, op.bitwise_and)
            nc.vector.tensor_tensor(v3(bma3), v3(bma3), bm_c, op.bitwise_and)

            # vertical shifts via SBUF->SBUF DMA (gpsimd SWDGE), wave-wise
            zrow = mk.tile([P, W], i16, name="zrow")
            nc.gpsimd.memset(zrow, 0)

            def vshift(s3, nm_dn, nm_up):
                dn = mk.tile([P, NF], i16, name=nm_dn, tag="sh", bufs=2)
                up = mk.tile([P, NF], i16, name=nm_up, tag="sh", bufs=2)
                d3, u3 = v3(dn), v3(up)
                nc.gpsimd.dma_start(d3[0:1, 0:1, :], zrow[0:1, :])
                nc.gpsimd.dma_start(d3[1:P, :, :], s3[0:P - 1, :, :])
                nc.gpsimd.dma_start(d3[0:1, 1:KB, :], s3[P - 1:P, 0:KB - 1, :])
                nc.gpsimd.dma_start(u3[P - 1:P, KB - 1:KB, :], zrow[0:1, :])
                nc.gpsimd.dma_start(u3[0:P - 1, :, :], s3[1:P, :, :])
                nc.gpsimd.dma_start(u3[P - 1:P, 0:KB - 1, :], s3[0:1, 1:KB])
                return dn, up

            # window combines on Pool, interleaved so scratch frees early
            odn, oup = vshift(v3(bmrow3), "odn", "oup")
            nc.vector.tensor_tensor(bmor, odn, oup, op.bitwise_or)
            nc.vector.tensor_tensor(bmor, bmor, bmrow3, op.bitwise_or)
            adn, aup = vshift(v3(bma3), "adn", "aup")
            nc.vector.tensor_tensor(bmand, adn, aup, op.bitwise_and)
            nc.vector.tensor_tensor(bmand, bmand, bma3, op.bitwise_and)
            bdn, bup = vshift(bm_c, "bdn", "bup")
            nc.vector.tensor_tensor(bm4, bdn, bup, op.bitwise_or)
            nc.vector.tensor_tensor(v3(bm4), v3(bm4), bm_l, op.bitwise_or)
            nc.vector.tensor_tensor(v3(bm4), v3(bm4), bm_r, op.bitwise_or)
            _mcm.__exit__(None, None, None)

            nc.vector.tensor_scalar(notbmand, bmand, 0x7FF, None,
                                    op.bitwise_xor)

            # denp: bit-extract then accumulate (accum reduce op must be
            # add/sub/mult/min/max, so the bitwise extract can't fuse)
            for c in range(1, C):
                d = yp.tile([P, NF], i16, name=f"dp{c}", tag="eqd", bufs=2)
                nc.vector.tensor_scalar(
                    d, bmor, c, 1, op.logical_shift_right, op.bitwise_and)
                d2 = yp.tile([P, NF], i16, name=f"dq{c}", tag="eqd", bufs=2)
                nc.vector.tensor_scalar(
                    d2, d, 1, 0.0, op.mult, op.add,
                    accum_out=stats[:, SC_DENP + c - 1:SC_DENP + c])

            # ---- phase A: stream logits (cast bf16), exp, Z accum ----
            _mxm = tc.tile_pool(name="mxpool", bufs=2)
            _lcm = tc.tile_pool(name="lpool", bufs=3)
            mxp = _mxm.__enter__()
            lpool = _lcm.__enter__()

            # running max chain (DVE is in-order; 2 rotating slots suffice)
            mx_run = [None]

            def fold_max(c):
                if c == 0:
                    return
                if c == 1:
                    a, b = E(0), E(1)
                else:
                    a, b = mx_run[0], E(c)
                o = emax if c == C - 1 else mxp.tile(
                    [P, NF], bf16, name=f"mx{c}", tag="mx", bufs=2)
                nc.vector.tensor_tensor(o, a, b, op.max)
                mx_run[0] = o

            with tc.tile_pool(name="zpsum", bufs=1, space="PSUM") as zp:
                zps = zp.tile([P, NF], f32, name="zps")
                for c in range(C):
                    lbc = lpool.tile([P, NF], bf16, name=f"lb{c}", tag="lb")
                    nc.gpsimd.dma_start(
                        lbc.rearrange("p (k w) -> p k w", w=W), xv[c])
                    nc.scalar.activation(E(c), lbc, act.Exp)
                    for k in range(KB):
                        nc.tensor.matmul(
                            zps[:, k * W:(k + 1) * W],
                            identb,
                            E(c)[:, k * W:(k + 1) * W],
                            start=(c == 0),
                            stop=(c == C - 1),
                        )
                    fold_max(c)
                _lcm.__exit__(None, None, None)
                lnz = cp.tile([P, NF], f32, name="lnz", tag="lnz", bufs=1)
                nc.scalar.activation(lnz, zps, act.Ln)
            nc.scalar.activation(r, lnz, act.Exp, scale=-1.0)
            _mxm.__exit__(None, None, None)

            # denn: ACT sign-telescope on bmand (fills ACT gap after r)
            for c in range(1, C):
                sd = yp.tile([P, NF], bf16, name=f"sd{c}", tag="sgd", bufs=1)
                nc.scalar.activation(
                    sd, bmand, act.Sign, bias=bsg[:, c - 1:c],
                    accum_out=statsa[:, c - 1:c])

            # ---- PB: multi-hot pred bitmask via scaled-identity matmuls ----
            with tc.tile_pool(name="pbpsum", bufs=1, space="PSUM") as pbp:
                pbps = pbp.tile([P, NF], f32, name="pbps")
                for c in range(1, C):
                    pm = cp.tile([P, NF], bf16, name=f"pm{c}", tag="pm", bufs=2)
                    nc.vector.tensor_tensor(pm, E(c), emax, op.is_ge)
                    for k in range(KB):
                        nc.tensor.matmul(
                            pbps[:, k * W:(k + 1) * W],
                            sid(c),
                            pm[:, k * W:(k + 1) * W],
                            start=(c == 1),
                            stop=(c == C - 1),
                        )
                pb16 = pp.tile([P, NF], i16, name="pb16")
                nc.vector.tensor_copy(pb16, pbps)

            # PB1 = PB & -PB: exact first-max one-hot
            pb1 = pp.tile([P, NF], i16, name="pb1")
            negpb = cp.tile([P, NF], i16, name="negpb", tag="sci", bufs=3)
            nc.vector.tensor_scalar(negpb, pb16, -1, None, op.mult)
            nc.vector.tensor_tensor(pb1, pb16, negpb, op.bitwise_and)

            # value-coded planes (Pool)
            vA = pp.tile([P, NF], i16, name="vA")
            vB3 = pp.tile([P, NF], i16, name="vB3")
            vB4 = pp.tile([P, NF], i16, name="vB4")
            xorbp = cp.tile([P, NF], i16, name="xorbp", tag="sci", bufs=3)
            nc.vector.tensor_tensor(v3(xorbp), bm_c, v3(pb1), op.bitwise_xor)
            y1p = cp.tile([P, NF], i16, name="y1p", tag="sci", bufs=3)
            nc.vector.tensor_tensor(v3(y1p), bm_c, v3(xorbp), op.bitwise_and)
            nc.vector.tensor_tensor(vA, y1p, notbmand, op.bitwise_and)
            vB = cp.tile([P, NF], i16, name="vB", tag="sci", bufs=3)
            nc.vector.tensor_tensor(vB, pb1, xorbp, op.bitwise_and)
            nc.vector.tensor_tensor(vB3, vB, bmor, op.bitwise_and)
            nc.vector.tensor_tensor(vB4, vB, bm4, op.bitwise_and)

            # ---- phase C: per-class softmax products (DVE+PE) ----
            with (
                tc.tile_pool(name="ptpsum", bufs=1, space="PSUM") as ptp,
                tc.tile_pool(name="hpsum", bufs=1, space="PSUM") as hp,
            ):
                ptps = ptp.tile([P, NF], f32, name="ptps")
                spbank = hp.tile([P, W], f32, name="spbank")
                inbank = hp.tile([P, W], f32, name="inbank")
                for c in range(C):
                    oh = cp.tile([P, NF], bf16, name=f"oh{c}", tag="oh", bufs=2)
                    nc.vector.tensor_scalar(
                        oh, t16i, c, 0.0, op.is_equal, op.add,
                        accum_out=stats[:, SC_COUNT + c:SC_COUNT + c + 1])
                    pc = cp.tile([P, NF], bf16, name=f"pc{c}", tag="pc", bufs=2)
                    nc.vector.tensor_tensor(pc, E(c), r, op.mult)
                    ohp = cp.tile([P, NF], bf16, name=f"ohp{c}", tag="ohp",
                                  bufs=2)
                    nc.vector.tensor_tensor(ohp, oh, pc, op.mult)
                    for k in range(KB):
                        sl = slice(k * W, (k + 1) * W)
                        nc.tensor.matmul(
                            spbank[:, :], ocol(c), pc[:, sl],
                            start=(c == 0 and k == 0),
                            stop=(c == C - 1 and k == KB - 1))
                        nc.tensor.matmul(
                            inbank[:, :], ocol(c), ohp[:, sl],
                            start=(c == 0 and k == 0),
                            stop=(c == C - 1 and k == KB - 1))
                        nc.tensor.matmul(
                            ptps[:, sl], identb, ohp[:, sl],
                            start=(c == 0),
                            stop=(c == C - 1))
                # per-class sums live in partition rows of the banks
                sp_sc = cp.tile([P, W], f32, name="sp_sc", tag="spsc", bufs=1)
                nc.vector.tensor_scalar(
                    sp_sc, spbank, 1, 0.0, op.mult, op.add,
                    accum_out=statsp[:, 0:1])
                in_sc = cp.tile([P, W], f32, name="in_sc", tag="insc", bufs=1)
                nc.vector.tensor_scalar(
                    in_sc, inbank, 1, 0.0, op.mult, op.add,
                    accum_out=statsp[:, 1:2])

                # focal from pt PSUM (ACT), product+accum on DVE
                lg = cp.tile([P, NF], bf16, name="lg", tag="lg", bufs=1)
                nc.scalar.activation(lg, ptps, act.Ln)
                q2 = cp.tile([P, NF], bf16, name="q2", tag="q2", bufs=1)
                nc.scalar.activation(q2, ptps, act.Square, bias=bq, scale=-1.0)
            fsc = cp.tile([P, NF], bf16, name="fsc", tag="fsc", bufs=1)
            nc.vector.scalar_tensor_tensor(
                fsc, q2, 1.0, lg, op.mult, op.mult,
                accum_out=stats[:, SC_FOCAL:SC_FOCAL + 1])

            # ---- value-coded eq counting (DVE) ----
            def eq_acc(src, val, col, nm):
                o = yp.tile([P, NF], i16, name=nm, tag="eqd", bufs=2)
                nc.vector.tensor_scalar(
                    o, src, val, 0.0, op.is_equal, op.add,
                    accum_out=stats[:, col:col + 1])

            for c in range(1, C):
                eq_acc(vA, 1 << c, SC_VA + c - 1, f"eva{c}")
                eq_acc(vB3, 1 << c, SC_VB3 + c - 1, f"evb3{c}")
                eq_acc(vB4, 1 << c, SC_VB4 + c - 1, f"evb4{c}")

            nc.gpsimd.dma_start(stats_out[:, :], stats)
            nc.gpsimd.dma_start(statsa_out[:, :], statsa)
            nc.gpsimd.dma_start(statsp_out[:, :], statsp)

    nc.compile()
    return nc


def _decode(res_list):
    """res_list: 8 dicts of arrays -> (total, dice, focal, edge)."""
    dices, focals, edges = [], [], []
    for rr in res_list:
        v = rr["stats"].astype(np.float64).sum(axis=0)
        sa = rr["statsa"].astype(np.float64).sum(axis=0)
        spv = rr["statsp"].astype(np.float64)
        count = v[SC_COUNT:SC_COUNT + 11]
        sump = spv[0:11, 0]
        inter = spv[0:11, 1]
        dice = (2.0 * inter + EPS) / (sump + count + EPS)
        dices.append(dice.mean())
        focals.append(-0.25 * v[SC_FOCAL] / NPIX)
        y1m2 = v[SC_VA:SC_VA + 10]
        ny3 = v[SC_VB3:SC_VB3 + 10]
        ny4 = v[SC_VB4:SC_VB4 + 10]
        denp = v[SC_DENP:SC_DENP + 10]
        # denn via sign-telescope: S_c = 2*M_c + n_c - NPIX, M_c = sum_{k>c} n_k
        denn = np.zeros(10)
        M = 0.0
        for c in range(10, 0, -1):
            n_c = sa[c - 1] + NPIX - 2.0 * M
            denn[c - 1] = n_c
            M += n_c
        num = y1m2 + ES2 * ny3 + (E1 - ES2) * ny4
        den = denp - denn
        cls = np.where(den > 0, num / np.maximum(den, 1.0), 0.0)
        edges.append(cls.mean())
    dice_loss = 1.0 - float(np.mean(dices))
    focal_loss = float(np.mean(focals))
    edge_loss = float(np.mean(edges))
    total = dice_loss + focal_loss + edge_loss
    return (
        np.float32(total),
        np.float32(dice_loss),
        np.float32(focal_loss),
        np.float32(edge_loss),
    )


def kernel(inputs: np.ndarray, targets: np.ndarray):
    from concourse.bass_utils import run_bass_kernel_spmd

    if "nc" not in _cache:
        _cache["nc"] = _build()
    nc = _cache["nc"]

    inputs = np.ascontiguousarray(np.asarray(inputs, dtype=np.float32))
    targets = np.ascontiguousarray(np.asarray(targets, dtype=np.int32))
    in_maps = [{"x": inputs[b], "t": targets[b]} for b in range(B)]
    res = run_bass_kernel_spmd(nc, in_maps, core_ids=list(range(B)))
    _cache["last_result"] = res
    return _decode(res.results)


# revision 21
# speedup vs baseline: 1.3290x; 1.0196x over previous
"""CompoundLoss (dice + focal + edge) Trainium2 Bass kernel, v2.

Self-contained: hardcodes shapes [8,11,512,512] f32 logits + [8,512,512] i32
targets, shards batch across 8 NeuronCores (pure data parallel). Each core
reduces its image to a few fp32 accumulator columns; the host finishes the
tiny scalar math in fp64.

v2 structure (vs v1): logits stream in as bf16 via casting SWDGE DMAs (halves
DMA bytes into SBUF); target-side mask pipeline (3x3 OR/AND windows via
shifted column views + partition-shift SBUF-SBUF DMAs) overlaps the logit
stream; exact one-hot argmax bitmask PB1 = PB & -PB (lowest set bit = first
maximal class, matching np.argmax tie behavior); edge counts via value-coded
planes + fused is_equal accumulations; eroded (bmand) counts via ACT
sign-telescope; focal fused on ACT directly from the pt PSUM.

Math notes (per image):
  softmax: E_c = exp(L_c) (bf16), Z = sum_c E_c (PE identity-matmul accumulate
  into PSUM, fp32), r = exp(-ln Z). pc = E_c*r, pt = sum_c [T==c]*pc (PE).
  dice: inter[c]/sumP[c] = colsums of ohp/pc via ocol matmuls into PSUM bank
  partition-rows, card[c] = sumP[c] + count[c] + eps.
  focal: mean(-0.25*(1-pt)^2*ln(pt)) with (1-pt)^2 and ln(pt) on ACT.
  edge: bm = 1<<T (i16), PB = sum 2^c*[E_c >= max E] (PE scaled-identity),
  PB1 = PB & -PB. 3x3 windows: row or/and via shifted column views, vertical
  via partition-shift SBUF-SBUF DMAs. Value-coded planes (one-hot-or-zero):
    vA  = bm & (bm^PB1) & ~bmand   -> count[vA==2^c]  = y1-y2
    vB3 = PB1 & (bm^PB1) & bmor    -> count[vB3==2^c] = y3
    vB4 = PB1 & (bm^PB1) & bm4     -> count[vB4==2^c] = y4
    denp[c] = popcount of bit c of bmor (fused shift+and+accum)
    denn[c] via ACT sign-telescope on bmand: S_c = 2*M_c + n_c - NPIX
"""

import numpy as np

B, C, H, W = 8, 11, 512, 512
P = 128
KB = H // P          # 4 row-blocks
NF = KB * W          # 2048 free elems per partition
NPIX = H * W
EPS = 1e-6
E1 = float(np.exp(-1.0))
ES2 = float(np.exp(-np.sqrt(2.0)))

# stats column layout (summed over partitions on host)
SC_COUNT = 0         # 11
SC_FOCAL = 11        # 1
SC_VA = 12           # 10 (c=1..10): y1 - y2
SC_VB3 = 22          # 10: y3
SC_VB4 = 32          # 10: y4
SC_DENP = 42         # 11: M_1..M_10 mod-sums + M_11 full sum
NCOL = 54
NACOL = 12           # statsa: denn sign-telescope S_c in cols 0..9

_cache = {}


def _build():
    import ml_dtypes
    import concourse.bacc as bacc
    import concourse.mybir as mybir
    from concourse.tile import TileContext

    f32 = mybir.dt.float32
    bf16 = mybir.dt.bfloat16
    i32 = mybir.dt.int32
    i16 = mybir.dt.int16
    op = mybir.AluOpType
    act = mybir.ActivationFunctionType

    nc = bacc.Bacc()
    x = nc.dram_tensor("x", [C, H, W], f32, kind="ExternalInput")
    t = nc.dram_tensor("t", [H, W], i32, kind="ExternalInput")
    stats_out = nc.dram_tensor("stats", [P, NCOL], f32, kind="ExternalOutput")
    statsa_out = nc.dram_tensor("statsa", [P, NACOL], f32, kind="ExternalOutput")
    statsp_out = nc.dram_tensor("statsp", [P, 4], f32, kind="ExternalOutput")

    # [C, 128, 4, 512] view: row = 128*k + p
    xv = x[:, :, :].rearrange("c (k p) w -> c p k w", p=P)
    tv = t[:, :].rearrange("(k p) w -> p k w", p=P)

    # constants: ident | sid(1..10) | ocol(0..10), one DMA
    ident_np = np.eye(P, dtype=np.float32)
    blocks = [ident_np]
    for c in range(1, C):
        blocks.append(ident_np * float(1 << c))
    ocol_np = np.zeros((C, P, P), dtype=np.float32)
    for c in range(C):
        ocol_np[c, :, c] = 1.0
    blocks.extend(ocol_np)
    ocol2_np = np.zeros((C, P, P), dtype=np.float32)
    for c in range(C):
        ocol2_np[c, :, 16 + c] = 1.0
    blocks.extend(ocol2_np)
    cb_np = np.concatenate(blocks, axis=1)           # [128, 33*128]
    cb_d = nc.inline_tensor(cb_np.astype(ml_dtypes.bfloat16), name="cb")
    zd = nc.inline_tensor(np.zeros((1, W), dtype=np.int16), name="zd")

    with TileContext(nc, pool_alloc_mode="queue") as tc:
        with (
            tc.tile_pool(name="persist", bufs=1) as pp,
            tc.tile_pool(name="cpool", bufs=2) as cp,
            tc.tile_pool(name="ypool", bufs=4) as yp,
        ):
            cbt = pp.tile([P, 33 * P], bf16, name="cbt")
            nc.sync.dma_start(cbt, cb_d[:, :])
            identb = cbt[:, 0:P]
            sid = lambda c: cbt[:, c * P:(c + 1) * P]            # c=1..10
            ocol = lambda c: cbt[:, (11 + c) * P:(12 + c) * P]   # c=0..10
            ocol2 = lambda c: cbt[:, (22 + c) * P:(23 + c) * P]  # c=0..10

            # targets: cast DMA i32 -> i16
            t16i = pp.tile([P, NF], i16, name="t16i")
            nc.gpsimd.dma_start(t16i.rearrange("p (k w) -> p k w", w=W), tv)
            t16i3 = t16i.rearrange("p (k w) -> p k w", w=W)

            stats = pp.tile([P, NCOL], f32, name="stats")
            statsa = pp.tile([P, NACOL], f32, name="statsa")
            statsp = pp.tile([P, 4], f32, name="statsp")

            # bias constants for ACT (per-partition scalars)
            bq = pp.tile([P, 1], f32, name="bq")
            nc.gpsimd.memset(bq, 1.0)
            bsg = pp.tile([P, 10], f32, name="bsg")
            for c in range(1, C):
                nc.gpsimd.memset(bsg[:, c - 1:c], -float(1 << c))

            # persistent mask planes
            NPAD = KB * (W + 2)
            bmp = pp.tile([P, NPAD], i16, name="bmp")
            bmp3 = bmp.rearrange("p (k w) -> p k w", w=W + 2)
            bm_c = bmp3[:, :, 1:W + 1]
            bm_l = bmp3[:, :, 0:W]
            bm_r = bmp3[:, :, 2:W + 2]
            bmor = pp.tile([P, NF], i16, name="bmor")
            bmand = pp.tile([P, NF], i16, name="bmand")
            bm4 = pp.tile([P, NF], i16, name="bm4")
            notbmand = pp.tile([P, NF], i16, name="notbmand")
            emax = pp.tile([P, NF], bf16, name="emax")
            r = pp.tile([P, NF], bf16, name="r")
            Et = pp.tile([P, C * NF], bf16, name="Et")
            E = lambda c: Et[:, c * NF:(c + 1) * NF]

            v3 = lambda a: a.rearrange("p (k w) -> p k w", w=W)

            # ---- target bitmask bm + 3x3 windows ----
            nc.gpsimd.memset(bmp3[:, :, 0:1], 0)
            nc.gpsimd.memset(bmp3[:, :, W + 1:W + 2], 0)
            _mcm = tc.tile_pool(name="maskp", bufs=1)
            mk = _mcm.__enter__()
            ones16 = mk.tile([P, NF], i16, name="ones16")
            nc.gpsimd.memset(ones16, 1)
            nc.vector.tensor_tensor(bm_c, v3(ones16), t16i3,
                                    op.logical_shift_left)
            # row windows (DVE)
            bmrow3 = mk.tile([P, NF], i16, name="bmrow3")
            nc.vector.tensor_tensor(v3(bmrow3), bm_l, bm_r, op.bitwise_or)
            nc.vector.tensor_tensor(v3(bmrow3), v3(bmrow3), bm_c, op.bitwise_or)
            bma3 = mk.tile([P, NF], i16, name="bma3")
            nc.vector.tensor_tensor(v3(bma3), bm_l, bm_r, op.bitwise_and)
            nc.vector.tensor_tensor(v3(bma3), v3(bma3), bm_c, op.bitwise_and)

            # vertical shifts via SBUF->SBUF DMA on the idle SP/HWDGE queue
            def vshift(s3, nm_dn, nm_up):
                dn = mk.tile([P, NF], i16, name=nm_dn, tag="sh", bufs=2)
                up = mk.tile([P, NF], i16, name=nm_up, tag="sh", bufs=2)
                d3, u3 = v3(dn), v3(up)
                nc.sync.dma_start(d3[0:1, 0:1, :], zd[:, :])
                nc.sync.dma_start(d3[1:P, :, :], s3[0:P - 1, :, :])
                nc.sync.dma_start(d3[0:1, 1:KB, :], s3[P - 1:P, 0:KB - 1, :])
                nc.sync.dma_start(u3[P - 1:P, KB - 1:KB, :], zd[:, :])
                nc.sync.dma_start(u3[0:P - 1, :, :], s3[1:P, :, :])
                nc.sync.dma_start(u3[P - 1:P, 0:KB - 1, :], s3[0:1, 1:KB])
                return dn, up

            # window combines on Pool, interleaved so scratch frees early
            odn, oup = vshift(v3(bmrow3), "odn", "oup")
            nc.vector.tensor_tensor(bmor, odn, oup, op.bitwise_or)
            nc.vector.tensor_tensor(bmor, bmor, bmrow3, op.bitwise_or)
            adn, aup = vshift(v3(bma3), "adn", "aup")
            nc.vector.tensor_tensor(bmand, adn, aup, op.bitwise_and)
            nc.vector.tensor_tensor(bmand, bmand, bma3, op.bitwise_and)
            bdn, bup = vshift(bm_c, "bdn", "bup")
            nc.vector.tensor_tensor(bm4, bdn, bup, op.bitwise_or)
            nc.vector.tensor_tensor(v3(bm4), v3(bm4), bm_l, op.bitwise_or)
            nc.vector.tensor_tensor(v3(bm4), v3(bm4), bm_r, op.bitwise_or)
            _mcm.__exit__(None, None, None)

            nc.vector.tensor_scalar(notbmand, bmand, 0x7FF, None,
                                    op.bitwise_xor)

            # denp: bit-extract then count (op0/op1 must share ALU class,
            # so the extract and the accumulating count are separate ops)
            for c in range(1, C):
                d = yp.tile([P, NF], i16, name=f"dp{c}", tag="eqd", bufs=2)
                nc.vector.tensor_scalar(
                    d, bmor, c, 1, op.logical_shift_right, op.bitwise_and)
                d2 = yp.tile([P, NF], i16, name=f"dq{c}", tag="eqd", bufs=2)
                nc.vector.tensor_scalar(
                    d2, d, 1, 0.0, op.mult, op.add,
                    accum_out=stats[:, SC_DENP + c - 1:SC_DENP + c])

            # ---- phase A: stream logits (cast bf16), exp, Z accum ----
            _mxm = tc.tile_pool(name="mxpool", bufs=2)
            _lcm = tc.tile_pool(name="lpool", bufs=3)
            mxp = _mxm.__enter__()
            lpool = _lcm.__enter__()

            # running max chain (DVE is in-order; 2 rotating slots suffice)
            mx_run = [None]

            def fold_max(c):
                if c == 0:
                    return
                if c == 1:
                    a, b = E(0), E(1)
                else:
                    a, b = mx_run[0], E(c)
                o = emax if c == C - 1 else mxp.tile(
                    [P, NF], bf16, name=f"mx{c}", tag="mx", bufs=2)
                nc.vector.tensor_tensor(o, a, b, op.max)
                mx_run[0] = o

            with tc.tile_pool(name="zpsum", bufs=1, space="PSUM") as zp:
                zps = zp.tile([P, NF], f32, name="zps")
                for c in range(C):
                    lbc = lpool.tile([P, NF], bf16, name=f"lb{c}", tag="lb")
                    nc.gpsimd.dma_start(
                        lbc.rearrange("p (k w) -> p k w", w=W), xv[c])
                    nc.scalar.activation(E(c), lbc, act.Exp)
                    for k in range(KB):
                        nc.tensor.matmul(
                            zps[:, k * W:(k + 1) * W],
                            identb,
                            E(c)[:, k * W:(k + 1) * W],
                            start=(c == 0),
                            stop=(c == C - 1),
                        )
                    fold_max(c)
                _lcm.__exit__(None, None, None)
                lnz = cp.tile([P, NF], f32, name="lnz", tag="lnz", bufs=1)
                nc.scalar.activation(lnz, zps, act.Ln)
            nc.scalar.activation(r, lnz, act.Exp, scale=-1.0)
            _mxm.__exit__(None, None, None)

            # denn: ACT sign-telescope on bmand (fills ACT gap after r)
            for c in range(1, C):
                sd = yp.tile([P, NF], bf16, name=f"sd{c}", tag="sgd", bufs=1)
                nc.scalar.activation(
                    sd, bmand, act.Sign, bias=bsg[:, c - 1:c],
                    accum_out=statsa[:, c - 1:c])

            # ---- fused PB + phase C ----
            # PB accumulates in two column-half PSUM pools (2 banks each) so
            # it coexists with ptps (4 banks) + the merged sp/in bank (1).
            # pm compares run on Pool in column halves; PB matmuls interleave
            # with phase C matmuls in PE issue order.
            NH = NF // 2
            pb16 = pp.tile([P, NF], i16, name="pb16")

            def pm_half(cc, h):
                sl = slice(h * NH, h * NH + NH)
                pmh = cp.tile([P, NH], bf16, name=f"pm{h}_{cc}", tag="pmh",
                              bufs=2)
                nc.vector.tensor_tensor(pmh, E(cc)[:, sl], emax[:, sl],
                                        op.is_ge)
                return pmh

            def pb_mms(pbps, pmh, cc, h):
                for k in range(2):
                    nc.tensor.matmul(
                        pbps[:, k * W:(k + 1) * W],
                        sid(cc),
                        pmh[:, k * W:(k + 1) * W],
                        start=(cc == 1),
                        stop=(cc == C - 1),
                    )

            with (
                tc.tile_pool(name="ptpsum", bufs=1, space="PSUM") as ptp,
                tc.tile_pool(name="hpsum", bufs=1, space="PSUM") as hp,
            ):
                ptps = ptp.tile([P, NF], f32, name="ptps")
                spin = hp.tile([P, W], f32, name="spin")
                _pbh = tc.tile_pool(name="pbA", bufs=1, space="PSUM")
                pbh = _pbh.__enter__()
                pbps = pbh.tile([P, NH], f32, name="pbpsA")
                for c in range(C):
                    if c <= 4:          # half A: pred classes 2c+1, 2c+2
                        for cc in (2 * c + 1, 2 * c + 2):
                            pb_mms(pbps, pm_half(cc, 0), cc, 0)
                    if c == 5:          # half A done -> copy out, swap pools
                        nc.vector.tensor_copy(pb16[:, 0:NH], pbps)
                        _pbh.__exit__(None, None, None)
                        _pbh = tc.tile_pool(name="pbB", bufs=1, space="PSUM")
                        pbh = _pbh.__enter__()
                        pbps = pbh.tile([P, NH], f32, name="pbpsB")
                    if 5 <= c <= 9:     # half B: pred classes 2(c-5)+1, +2
                        for cc in (2 * (c - 5) + 1, 2 * (c - 5) + 2):
                            pb_mms(pbps, pm_half(cc, 1), cc, 1)
                    oh = cp.tile([P, NF], bf16, name=f"oh{c}", tag="oh", bufs=2)
                    nc.vector.tensor_scalar(
                        oh, t16i, c, 0.0, op.is_equal, op.add,
                        accum_out=stats[:, SC_COUNT + c:SC_COUNT + c + 1])
                    pc = cp.tile([P, NF], bf16, name=f"pc{c}", tag="pc", bufs=2)
                    nc.vector.tensor_tensor(pc, E(c), r, op.mult)
                    ohp = cp.tile([P, NF], bf16, name=f"ohp{c}", tag="ohp",
                                  bufs=2)
                    nc.vector.tensor_tensor(ohp, oh, pc, op.mult)
                    for k in range(KB):
                        sl = slice(k * W, (k + 1) * W)
                        nc.tensor.matmul(
                            ptps[:, sl], identb, ohp[:, sl],
                            start=(c == 0),
                            stop=(c == C - 1))
                        nc.tensor.matmul(
                            spin[:, :], ocol(c), pc[:, sl],
                            start=(c == 0 and k == 0),
                            stop=False)
                        nc.tensor.matmul(
                            spin[:, :], ocol2(c), ohp[:, sl],
                            start=False,
                            stop=(c == C - 1 and k == KB - 1))
                nc.vector.tensor_copy(pb16[:, NH:NF], pbps)
                _pbh.__exit__(None, None, None)

                # PB1 = PB & -PB: exact first-max one-hot
                pb1 = pp.tile([P, NF], i16, name="pb1")
                negpb = cp.tile([P, NF], i16, name="negpb", tag="sci", bufs=3)
                nc.vector.tensor_scalar(negpb, pb16, -1, None, op.mult)
                nc.vector.tensor_tensor(pb1, pb16, negpb, op.bitwise_and)

                # value-coded planes (DVE; bitwise is DVE-only)
                vA = pp.tile([P, NF], i16, name="vA")
                vB3 = pp.tile([P, NF], i16, name="vB3")
                vB4 = pp.tile([P, NF], i16, name="vB4")
                xorbp = cp.tile([P, NF], i16, name="xorbp", tag="sci", bufs=3)
                nc.vector.tensor_tensor(v3(xorbp), bm_c, v3(pb1),
                                        op.bitwise_xor)
                y1p = cp.tile([P, NF], i16, name="y1p", tag="sci", bufs=3)
                nc.vector.tensor_tensor(v3(y1p), bm_c, v3(xorbp),
                                        op.bitwise_and)
                nc.vector.tensor_tensor(vA, y1p, notbmand, op.bitwise_and)
                vB = cp.tile([P, NF], i16, name="vB", tag="sci", bufs=3)
                nc.vector.tensor_tensor(vB, pb1, xorbp, op.bitwise_and)
                nc.vector.tensor_tensor(vB3, vB, bmor, op.bitwise_and)
                nc.vector.tensor_tensor(vB4, vB, bm4, op.bitwise_and)

                # eq counting (DVE) while PE drains the bank matmuls
                def eq_acc(src_, val, col, nm):
                    o = yp.tile([P, NF], i16, name=nm, tag="eqd", bufs=2)
                    nc.vector.tensor_scalar(
                        o, src_, val, 0.0, op.is_equal, op.add,
                        accum_out=stats[:, col:col + 1])

                for c in range(1, C):
                    eq_acc(vA, 1 << c, SC_VA + c - 1, f"eva{c}")
                    eq_acc(vB3, 1 << c, SC_VB3 + c - 1, f"evb3{c}")
                    eq_acc(vB4, 1 << c, SC_VB4 + c - 1, f"evb4{c}")

                # merged bank reduce: rows 0-10 = sumP, rows 16-26 = inter
                sp_sc = cp.tile([P, W], f32, name="sp_sc", tag="spsc", bufs=1)
                nc.vector.tensor_scalar(
                    sp_sc, spin, 1, 0.0, op.mult, op.add,
                    accum_out=statsp[:, 0:1])

                # focal from pt PSUM (ACT), product+accum on DVE
                lg = cp.tile([P, NF], bf16, name="lg", tag="lg", bufs=1)
                nc.scalar.activation(lg, ptps, act.Ln)
                q2 = cp.tile([P, NF], bf16, name="q2", tag="q2", bufs=1)
                nc.scalar.activation(q2, ptps, act.Square, bias=bq, scale=-1.0)
            fsc = cp.tile([P, NF], bf16, name="fsc", tag="fsc", bufs=1)
            nc.vector.scalar_tensor_tensor(
                fsc, q2, 1.0, lg, op.mult, op.mult,
                accum_out=stats[:, SC_FOCAL:SC_FOCAL + 1])

            nc.gpsimd.dma_start(stats_out[:, :], stats)
            nc.gpsimd.dma_start(statsa_out[:, :], statsa)
            nc.gpsimd.dma_start(statsp_out[:, :], statsp)

    nc.compile()
    return nc


def _decode(res_list):
    """res_list: 8 dicts of arrays -> (total, dice, focal, edge)."""
    dices, focals, edges = [], [], []
    for rr in res_list:
        v = rr["stats"].astype(np.float64).sum(axis=0)
        sa = rr["statsa"].astype(np.float64).sum(axis=0)
        spv = rr["statsp"].astype(np.float64)
        count = v[SC_COUNT:SC_COUNT + 11]
        sump = spv[0:11, 0]
        inter = spv[16:27, 0]
        dice = (2.0 * inter + EPS) / (sump + count + EPS)
        dices.append(dice.mean())
        focals.append(-0.25 * v[SC_FOCAL] / NPIX)
        y1m2 = v[SC_VA:SC_VA + 10]
        ny3 = v[SC_VB3:SC_VB3 + 10]
        ny4 = v[SC_VB4:SC_VB4 + 10]
        denp = v[SC_DENP:SC_DENP + 10]
        # denn via sign-telescope: S_c = 2*M_c + n_c - NPIX, M_c = sum_{k>c} n_k
        denn = np.zeros(10)
        M = 0.0
        for c in range(10, 0, -1):
            n_c = sa[c - 1] + NPIX - 2.0 * M
            denn[c - 1] = n_c
            M += n_c
        num = y1m2 + ES2 * ny3 + (E1 - ES2) * ny4
        den = denp - denn
        cls = np.where(den > 0, num / np.maximum(den, 1.0), 0.0)
        edges.append(cls.mean())
    dice_loss = 1.0 - float(np.mean(dices))
    focal_loss = float(np.mean(focals))
    edge_loss = float(np.mean(edges))
    total = dice_loss + focal_loss + edge_loss
    return (
        np.float32(total),
        np.float32(dice_loss),
        np.float32(focal_loss),
        np.float32(edge_loss),
    )


def kernel(inputs: np.ndarray, targets: np.ndarray):
    from concourse.bass_utils import run_bass_kernel_spmd

    if "nc" not in _cache:
        _cache["nc"] = _build()
    nc = _cache["nc"]

    inputs = np.ascontiguousarray(np.asarray(inputs, dtype=np.float32))
    targets = np.ascontiguousarray(np.asarray(targets, dtype=np.int32))
    in_maps = [{"x": inputs[b], "t": targets[b]} for b in range(B)]
    res = run_bass_kernel_spmd(nc, in_maps, core_ids=list(range(B)))
    _cache["last_result"] = res
    return _decode(res.results)


# revision 23
# speedup vs baseline: 1.4612x; 1.0994x over previous
"""CompoundLoss (dice + focal + edge) Trainium2 Bass kernel, v2.

Self-contained: hardcodes shapes [8,11,512,512] f32 logits + [8,512,512] i32
targets, shards batch across 8 NeuronCores (pure data parallel). Each core
reduces its image to a few fp32 accumulator columns; the host finishes the
tiny scalar math in fp64.

v2 structure (vs v1): logits stream in as bf16 via casting SWDGE DMAs (halves
DMA bytes into SBUF); target-side mask pipeline (3x3 OR/AND windows via
shifted column views + partition-shift SBUF-SBUF DMAs) overlaps the logit
stream; exact one-hot argmax bitmask PB1 = PB & -PB (lowest set bit = first
maximal class, matching np.argmax tie behavior); edge counts via value-coded
planes + fused is_equal accumulations; eroded (bmand) counts via ACT
sign-telescope; focal fused on ACT directly from the pt PSUM.

Math notes (per image):
  softmax: E_c = exp(L_c) (bf16), Z = sum_c E_c (PE identity-matmul accumulate
  into PSUM, fp32), r = exp(-ln Z). pc = E_c*r, pt = sum_c [T==c]*pc (PE).
  dice: inter[c]/sumP[c] = colsums of ohp/pc via ocol matmuls into PSUM bank
  partition-rows, card[c] = sumP[c] + count[c] + eps.
  focal: mean(-0.25*(1-pt)^2*ln(pt)) with (1-pt)^2 and ln(pt) on ACT.
  edge: bm = 1<<T (i16), PB = sum 2^c*[E_c >= max E] (PE scaled-identity),
  PB1 = PB & -PB. 3x3 windows: row or/and via shifted column views, vertical
  via partition-shift SBUF-SBUF DMAs. Value-coded planes (one-hot-or-zero):
    vA  = bm & (bm^PB1) & ~bmand   -> count[vA==2^c]  = y1-y2
    vB3 = PB1 & (bm^PB1) & bmor    -> count[vB3==2^c] = y3
    vB4 = PB1 & (bm^PB1) & bm4     -> count[vB4==2^c] = y4
    denp[c] = popcount of bit c of bmor (fused shift+and+accum)
    denn[c] via ACT sign-telescope on bmand: S_c = 2*M_c + n_c - NPIX
"""

import numpy as np

B, C, H, W = 8, 11, 512, 512
P = 128
KB = H // P          # 4 row-blocks
NF = KB * W          # 2048 free elems per partition
NPIX = H * W
EPS = 1e-6
E1 = float(np.exp(-1.0))
ES2 = float(np.exp(-np.sqrt(2.0)))

# stats column layout (summed over partitions on host)
SC_COUNT = 0         # 11
SC_FOCAL = 11        # 1
SC_VA = 12           # 10 (c=1..10): y1 - y2
SC_VB3 = 22          # 10: y3
SC_VB4 = 32          # 10: y4
SC_DENP = 42         # 11: M_1..M_10 mod-sums + M_11 full sum
NCOL = 54
NACOL = 12           # statsa: denn sign-telescope S_c in cols 0..9

_cache = {}


def _build():
    import ml_dtypes
    import concourse.bacc as bacc
    import concourse.mybir as mybir
    from concourse.tile import TileContext

    f32 = mybir.dt.float32
    bf16 = mybir.dt.bfloat16
    i32 = mybir.dt.int32
    i16 = mybir.dt.int16
    op = mybir.AluOpType
    act = mybir.ActivationFunctionType

    nc = bacc.Bacc()
    x = nc.dram_tensor("x", [C, H, W], f32, kind="ExternalInput")
    t = nc.dram_tensor("t", [H, W], i32, kind="ExternalInput")
    stats_out = nc.dram_tensor("stats", [P, NCOL], f32, kind="ExternalOutput")
    statsa_out = nc.dram_tensor("statsa", [P, NACOL], f32, kind="ExternalOutput")
    statsp_out = nc.dram_tensor("statsp", [P, 4], f32, kind="ExternalOutput")

    # [C, 128, 4, 512] view: row = 128*k + p
    xv = x[:, :, :].rearrange("c (k p) w -> c p k w", p=P)
    tv = t[:, :].rearrange("(k p) w -> p k w", p=P)

    # constants: ident | sid(1..10) | ocol(0..10), one DMA
    ident_np = np.eye(P, dtype=np.float32)
    blocks = [ident_np]
    for c in range(1, C):
        blocks.append(ident_np * float(1 << c))
    ocol_np = np.zeros((C, P, P), dtype=np.float32)
    for c in range(C):
        ocol_np[c, :, c] = 1.0
    blocks.extend(ocol_np)
    ocol2_np = np.zeros((C, P, P), dtype=np.float32)
    for c in range(C):
        ocol2_np[c, :, 16 + c] = 1.0
    blocks.extend(ocol2_np)
    cb_np = np.concatenate(blocks, axis=1)           # [128, 33*128]
    cb_d = nc.inline_tensor(cb_np.astype(ml_dtypes.bfloat16), name="cb")
    zd = nc.inline_tensor(np.zeros((1, W), dtype=np.int16), name="zd")

    with TileContext(nc, pool_alloc_mode="queue") as tc:
        with (
            tc.tile_pool(name="persist", bufs=1) as pp,
            tc.tile_pool(name="cpool", bufs=2) as cp,
            tc.tile_pool(name="ypool", bufs=4) as yp,
        ):
            cbt = pp.tile([P, 33 * P], bf16, name="cbt")
            nc.sync.dma_start(cbt, cb_d[:, :])
            identb = cbt[:, 0:P]
            sid = lambda c: cbt[:, c * P:(c + 1) * P]            # c=1..10
            ocol = lambda c: cbt[:, (11 + c) * P:(12 + c) * P]   # c=0..10
            ocol2 = lambda c: cbt[:, (22 + c) * P:(23 + c) * P]  # c=0..10

            # targets: cast DMA i32 -> i16
            t16i = pp.tile([P, NF], i16, name="t16i")
            nc.gpsimd.dma_start(t16i.rearrange("p (k w) -> p k w", w=W), tv)
            t16i3 = t16i.rearrange("p (k w) -> p k w", w=W)

            stats = pp.tile([P, NCOL], f32, name="stats")
            statsa = pp.tile([P, NACOL], f32, name="statsa")
            statsp = pp.tile([P, 4], f32, name="statsp")

            # bias constants for ACT (per-partition scalars)
            bq = pp.tile([P, 1], f32, name="bq")
            nc.gpsimd.memset(bq, 1.0)
            bsg = pp.tile([P, 10], f32, name="bsg")
            for c in range(1, C):
                nc.gpsimd.memset(bsg[:, c - 1:c], -float(1 << c))

            # persistent mask planes
            NPAD = KB * (W + 2)
            bmp = pp.tile([P, NPAD], i16, name="bmp")
            bmp3 = bmp.rearrange("p (k w) -> p k w", w=W + 2)
            bm_c = bmp3[:, :, 1:W + 1]
            bm_l = bmp3[:, :, 0:W]
            bm_r = bmp3[:, :, 2:W + 2]
            bmor = pp.tile([P, NF], i16, name="bmor")
            bmand = pp.tile([P, NF], i16, name="bmand")
            bm4 = pp.tile([P, NF], i16, name="bm4")
            emax = pp.tile([P, NF], bf16, name="emax")
            r = pp.tile([P, NF], bf16, name="r")
            Et = pp.tile([P, C * NF], bf16, name="Et")
            E = lambda c: Et[:, c * NF:(c + 1) * NF]

            v3 = lambda a: a.rearrange("p (k w) -> p k w", w=W)

            # ---- target bitmask bm + 3x3 windows ----
            nc.gpsimd.memset(bmp3[:, :, 0:1], 0)
            nc.gpsimd.memset(bmp3[:, :, W + 1:W + 2], 0)
            _mxm = tc.tile_pool(name="mxpool", bufs=2)
            _lcm = tc.tile_pool(name="lpool", bufs=2)
            mxp = _mxm.__enter__()
            lpool = _lcm.__enter__()
            _mcm = tc.tile_pool(name="maskp", bufs=1)
            mk = _mcm.__enter__()
            ones16 = mk.tile([P, NF], i16, name="ones16")
            nc.gpsimd.memset(ones16, 1)
            nc.vector.tensor_tensor(bm_c, v3(ones16), t16i3,
                                    op.logical_shift_left)
            # row windows (DVE)
            bmrow3 = mk.tile([P, NF], i16, name="bmrow3")
            nc.vector.tensor_tensor(v3(bmrow3), bm_l, bm_r, op.bitwise_or)
            nc.vector.tensor_tensor(v3(bmrow3), v3(bmrow3), bm_c, op.bitwise_or)
            bma3 = mk.tile([P, NF], i16, name="bma3")
            nc.vector.tensor_tensor(v3(bma3), bm_l, bm_r, op.bitwise_and)
            nc.vector.tensor_tensor(v3(bma3), v3(bma3), bm_c, op.bitwise_and)

            # vertical shifts via SBUF->SBUF DMA on the idle SP/HWDGE queue
            def vshift(s3, nm_dn, nm_up):
                dn = mk.tile([P, NF], i16, name=nm_dn, tag="sh", bufs=2)
                up = mk.tile([P, NF], i16, name=nm_up, tag="sh", bufs=2)
                d3, u3 = v3(dn), v3(up)
                nc.sync.dma_start(d3[0:1, 0:1, :], zd[:, :])
                nc.sync.dma_start(d3[1:P, :, :], s3[0:P - 1, :, :])
                nc.sync.dma_start(d3[0:1, 1:KB, :], s3[P - 1:P, 0:KB - 1, :])
                nc.sync.dma_start(u3[P - 1:P, KB - 1:KB, :], zd[:, :])
                nc.sync.dma_start(u3[0:P - 1, :, :], s3[1:P, :, :])
                nc.sync.dma_start(u3[P - 1:P, 0:KB - 1, :], s3[0:1, 1:KB])
                return dn, up

            # window combines on Pool, interleaved so scratch frees early
            odn, oup = vshift(v3(bmrow3), "odn", "oup")
            nc.vector.tensor_tensor(bmor, odn, oup, op.bitwise_or)
            nc.vector.tensor_tensor(bmor, bmor, bmrow3, op.bitwise_or)
            adn, aup = vshift(v3(bma3), "adn", "aup")
            nc.vector.tensor_tensor(bmand, adn, aup, op.bitwise_and)
            nc.vector.tensor_tensor(bmand, bmand, bma3, op.bitwise_and)
            bdn, bup = vshift(bm_c, "bdn", "bup")
            nc.vector.tensor_tensor(bm4, bdn, bup, op.bitwise_or)
            nc.vector.tensor_tensor(v3(bm4), v3(bm4), bm_l, op.bitwise_or)
            nc.vector.tensor_tensor(v3(bm4), v3(bm4), bm_r, op.bitwise_or)
            _mcm.__exit__(None, None, None)

            # denp: bit-extract then count (op0/op1 must share ALU class,
            # so the extract and the accumulating count are separate ops)
            for c in range(1, C):
                d = yp.tile([P, NF], i16, name=f"dp{c}", tag="eqd", bufs=2)
                nc.vector.tensor_scalar(
                    d, bmor, c, 1, op.logical_shift_right, op.bitwise_and)
                d2 = yp.tile([P, NF], i16, name=f"dq{c}", tag="eqd", bufs=2)
                nc.vector.tensor_scalar(
                    d2, d, 1, 0.0, op.mult, op.add,
                    accum_out=stats[:, SC_DENP + c - 1:SC_DENP + c])

            # ---- phase A: stream logits (cast bf16), exp, Z accum ----
            # running max chain (DVE is in-order; 2 rotating slots suffice)
            mx_run = [None]

            def fold_max(c):
                if c == 0:
                    return
                if c == 1:
                    a, b = E(0), E(1)
                else:
                    a, b = mx_run[0], E(c)
                o = emax if c == C - 1 else mxp.tile(
                    [P, NF], bf16, name=f"mx{c}", tag="mx", bufs=2)
                nc.vector.tensor_tensor(o, a, b, op.max)
                mx_run[0] = o

            with tc.tile_pool(name="zpsum", bufs=1, space="PSUM") as zp:
                zps = zp.tile([P, NF], f32, name="zps")
                for c in range(C):
                    lbc = lpool.tile([P, NF], bf16, name=f"lb{c}", tag="lb")
                    nc.gpsimd.dma_start(
                        lbc.rearrange("p (k w) -> p k w", w=W), xv[c])
                    nc.scalar.activation(E(c), lbc, act.Exp)
                    for k in range(KB):
                        nc.tensor.matmul(
                            zps[:, k * W:(k + 1) * W],
                            identb,
                            E(c)[:, k * W:(k + 1) * W],
                            start=(c == 0),
                            stop=(c == C - 1),
                        )
                    fold_max(c)
                _lcm.__exit__(None, None, None)
                lnz = cp.tile([P, NF], f32, name="lnz", tag="lnz", bufs=1)
                nc.scalar.activation(lnz, zps, act.Ln)
            nc.scalar.activation(r, lnz, act.Exp, scale=-1.0)
            _mxm.__exit__(None, None, None)

            # denn: ACT sign-telescope on bmand (fills ACT gap after r)
            for c in range(1, C):
                sd = yp.tile([P, NF], bf16, name=f"sd{c}", tag="sgd", bufs=1)
                nc.scalar.activation(
                    sd, bmand, act.Sign, bias=bsg[:, c - 1:c],
                    accum_out=statsa[:, c - 1:c])

            # ---- fused PB + phase C ----
            # PB accumulates in two column-half PSUM pools (2 banks each) so
            # it coexists with ptps (4 banks) + the merged sp/in bank (1).
            # pm compares run on Pool in column halves; PB matmuls interleave
            # with phase C matmuls in PE issue order.
            NH = NF // 2
            pb16 = pp.tile([P, NF], i16, name="pb16")

            def pm_half(cc, h):
                sl = slice(h * NH, h * NH + NH)
                pmh = cp.tile([P, NH], bf16, name=f"pm{h}_{cc}", tag="pmh",
                              bufs=2)
                nc.vector.tensor_tensor(pmh, E(cc)[:, sl], emax[:, sl],
                                        op.is_ge)
                return pmh

            def pb_mms(pbps, pmh, cc, h):
                for k in range(2):
                    nc.tensor.matmul(
                        pbps[:, k * W:(k + 1) * W],
                        sid(cc),
                        pmh[:, k * W:(k + 1) * W],
                        start=(cc == 1),
                        stop=(cc == C - 1),
                    )

            with (
                tc.tile_pool(name="ptpsum", bufs=1, space="PSUM") as ptp,
                tc.tile_pool(name="hpsum", bufs=1, space="PSUM") as hp,
            ):
                ptps = ptp.tile([P, NF], f32, name="ptps")
                spin = hp.tile([P, W], f32, name="spin")
                _pbh = tc.tile_pool(name="pbA", bufs=1, space="PSUM")
                pbh = _pbh.__enter__()
                pbps = pbh.tile([P, NH], f32, name="pbpsA")
                for c in range(C):
                    if c <= 4:          # half A: pred classes 2c+1, 2c+2
                        for cc in (2 * c + 1, 2 * c + 2):
                            pb_mms(pbps, pm_half(cc, 0), cc, 0)
                    if c == 5:          # half A done -> copy out, swap pools
                        nc.vector.tensor_copy(pb16[:, 0:NH], pbps)
                        _pbh.__exit__(None, None, None)
                        _pbh = tc.tile_pool(name="pbB", bufs=1, space="PSUM")
                        pbh = _pbh.__enter__()
                        pbps = pbh.tile([P, NH], f32, name="pbpsB")
                    if 5 <= c <= 9:     # half B: pred classes 2(c-5)+1, +2
                        for cc in (2 * (c - 5) + 1, 2 * (c - 5) + 2):
                            pb_mms(pbps, pm_half(cc, 1), cc, 1)
                    oh = cp.tile([P, NF], bf16, name=f"oh{c}", tag="oh", bufs=1)
                    nc.vector.tensor_scalar(
                        oh, t16i, c, 0.0, op.is_equal, op.add,
                        accum_out=stats[:, SC_COUNT + c:SC_COUNT + c + 1])
                    pc = cp.tile([P, NF], bf16, name=f"pc{c}", tag="pc", bufs=2)
                    nc.vector.tensor_tensor(pc, E(c), r, op.mult)
                    ohp = cp.tile([P, NF], bf16, name=f"ohp{c}", tag="ohp",
                                  bufs=2)
                    nc.vector.tensor_tensor(ohp, oh, pc, op.mult)
                    for k in range(KB):
                        sl = slice(k * W, (k + 1) * W)
                        nc.tensor.matmul(
                            ptps[:, sl], identb, ohp[:, sl],
                            start=(c == 0),
                            stop=(c == C - 1))
                        nc.tensor.matmul(
                            spin[:, :], ocol(c), pc[:, sl],
                            start=(c == 0 and k == 0),
                            stop=False)
                        nc.tensor.matmul(
                            spin[:, :], ocol2(c), ohp[:, sl],
                            start=False,
                            stop=(c == C - 1 and k == KB - 1))
                nc.vector.tensor_copy(pb16[:, NH:NF], pbps)
                _pbh.__exit__(None, None, None)

                # PB1 = PB & -PB: exact first-max one-hot
                pb1 = pp.tile([P, NF], i16, name="pb1")
                negpb = cp.tile([P, NF], i16, name="negpb", tag="sci", bufs=3)
                nc.vector.tensor_scalar(negpb, pb16, -1, None, op.mult)
                nc.vector.tensor_tensor(pb1, pb16, negpb, op.bitwise_and)

                # value-coded planes (DVE; bitwise is DVE-only)
                vA = pp.tile([P, NF], i16, name="vA")
                vB3 = pp.tile([P, NF], i16, name="vB3")
                vB4 = pp.tile([P, NF], i16, name="vB4")
                xorbp = cp.tile([P, NF], i16, name="xorbp", tag="sci", bufs=3)
                nc.vector.tensor_tensor(v3(xorbp), bm_c, v3(pb1),
                                        op.bitwise_xor)
                y1p = cp.tile([P, NF], i16, name="y1p", tag="sci", bufs=3)
                nc.vector.tensor_tensor(v3(y1p), bm_c, v3(xorbp),
                                        op.bitwise_and)
                y1t = cp.tile([P, NF], i16, name="y1t", tag="sci", bufs=3)
                nc.vector.tensor_tensor(y1t, y1p, bmand, op.bitwise_and)
                nc.vector.tensor_tensor(vA, y1p, y1t, op.bitwise_xor)
                vB = cp.tile([P, NF], i16, name="vB", tag="sci", bufs=3)
                nc.vector.tensor_tensor(vB, pb1, xorbp, op.bitwise_and)
                nc.vector.tensor_tensor(vB3, vB, bmor, op.bitwise_and)
                nc.vector.tensor_tensor(vB4, vB, bm4, op.bitwise_and)

                # eq counting (DVE) while PE drains the bank matmuls
                def eq_acc(src_, val, col, nm):
                    o = yp.tile([P, NF], i16, name=nm, tag="eqd", bufs=2)
                    nc.vector.tensor_scalar(
                        o, src_, val, 0.0, op.is_equal, op.add,
                        accum_out=stats[:, col:col + 1])

                for c in range(1, C):
                    eq_acc(vA, 1 << c, SC_VA + c - 1, f"eva{c}")
                    eq_acc(vB3, 1 << c, SC_VB3 + c - 1, f"evb3{c}")
                    eq_acc(vB4, 1 << c, SC_VB4 + c - 1, f"evb4{c}")

                # merged bank reduce: rows 0-10 = sumP, rows 16-26 = inter
                sp_sc = cp.tile([P, W], bf16, name="sp_sc", tag="spsc", bufs=1)
                nc.vector.tensor_scalar(
                    sp_sc, spin, 1, 0.0, op.mult, op.add,
                    accum_out=statsp[:, 0:1])

                # focal from pt PSUM (ACT), product+accum on DVE
                lg = cp.tile([P, NF], bf16, name="lg", tag="lg", bufs=2)
                nc.scalar.activation(lg, ptps, act.Ln)
                q2 = cp.tile([P, NF], bf16, name="q2", tag="q2", bufs=1)
                nc.scalar.activation(q2, ptps, act.Square, bias=bq, scale=-1.0)
            fsc = cp.tile([P, NF], bf16, name="fsc", tag="lg", bufs=2)
            nc.vector.scalar_tensor_tensor(
                fsc, q2, 1.0, lg, op.mult, op.mult,
                accum_out=stats[:, SC_FOCAL:SC_FOCAL + 1])

            nc.gpsimd.dma_start(stats_out[:, :], stats)
            nc.gpsimd.dma_start(statsa_out[:, :], statsa)
            nc.gpsimd.dma_start(statsp_out[:, :], statsp)

    nc.compile()
    return nc


def _decode(res_list):
    """res_list: 8 dicts of arrays -> (total, dice, focal, edge)."""
    dices, focals, edges = [], [], []
    for rr in res_list:
        v = rr["stats"].astype(np.float64).sum(axis=0)
        sa = rr["statsa"].astype(np.float64).sum(axis=0)
        spv = rr["statsp"].astype(np.float64)
        count = v[SC_COUNT:SC_COUNT + 11]
        sump = spv[0:11, 0]
        inter = spv[16:27, 0]
        dice = (2.0 * inter + EPS) / (sump + count + EPS)
        dices.append(dice.mean())
        focals.append(-0.25 * v[SC_FOCAL] / NPIX)
        y1m2 = v[SC_VA:SC_VA + 10]
        ny3 = v[SC_VB3:SC_VB3 + 10]
        ny4 = v[SC_VB4:SC_VB4 + 10]
        denp = v[SC_DENP:SC_DENP + 10]
        # denn via sign-telescope: S_c = 2*M_c + n_c - NPIX, M_c = sum_{k>c} n_k
        denn = np.zeros(10)
        M = 0.0
        for c in range(10, 0, -1):
            n_c = sa[c - 1] + NPIX - 2.0 * M
            denn[c - 1] = n_c
            M += n_c
        num = y1m2 + ES2 * ny3 + (E1 - ES2) * ny4
        den = denp - denn
        cls = np.where(den > 0, num / np.maximum(den, 1.0), 0.0)
        edges.append(cls.mean())
    dice_loss = 1.0 - float(np.mean(dices))
    focal_loss = float(np.mean(focals))
    edge_loss = float(np.mean(edges))
    total = dice_loss + focal_loss + edge_loss
    return (
        np.float32(total),
        np.float32(dice_loss),
        np.float32(focal_loss),
        np.float32(edge_loss),
    )


def kernel(inputs: np.ndarray, targets: np.ndarray):
    from concourse.bass_utils import run_bass_kernel_spmd

    if "nc" not in _cache:
        _cache["nc"] = _build()
    nc = _cache["nc"]

    inputs = np.ascontiguousarray(np.asarray(inputs, dtype=np.float32))
    targets = np.ascontiguousarray(np.asarray(targets, dtype=np.int32))
    in_maps = [{"x": inputs[b], "t": targets[b]} for b in range(B)]
    res = run_bass_kernel_spmd(nc, in_maps, core_ids=list(range(B)))
    _cache["last_result"] = res
    return _decode(res.results)
